# revision 7
# baseline (speedup 1.0000x reference)
# Masked-reset LSTM (MemoryEmbedding) Trainium2 kernel.
#
# Strategy: the episode-reset mask M (binary) splits every batch row into
# independent episodes, each starting from a known state (S[row] at t=0,
# zeros at every reset).  On the host we segment all B*T steps into
# episodes, sort them by length (desc) and deal them round-robin onto the
# 8 cores.  The device then runs a "wavefront" over relative step r
# (round r processes step r of every episode longer than r); because the
# episodes are sorted, the active set at every round is a prefix of the
# slot axis, so all device work is dense.  Sequential depth drops from
# T=2048 to max-episode-length (~56 for 10% reset rate).
#
# Device layout is fully transposed (gate/hidden units on partitions,
# episode slots on the free axis):
#   z^T[j*128:(j+1)*128, slot] = W1[:, j]^T @ obs1^T + R2[:, j]^T @ h^T
# with W1 = [W_embed @ kernel ; b_embed @ kernel + bias] (65 rows: 64 obs
# dims + ones-row for the bias) and gate columns permuted to [i|f|o|g] so
# one sigmoid covers blocks 0..5 and one tanh covers blocks 6..7.

import json

import ml_dtypes
import numpy as np

OBS_DIM, EMBED_DIM, HIDDEN = 64, 128, 256
BATCH, SEQLEN = 32, 2048
N_CORES = 8
GH = HIDDEN // 128  # 2 gate-partition tiles per gate

BF16 = ml_dtypes.bfloat16

# ---------------------------------------------------------------------------
# host-side helpers
# ---------------------------------------------------------------------------


def _episodes(M2):
    """M2: [B, T] binary mask -> arrays (row, start, length), unsorted."""
    rows, starts, lens = [], [], []
    for b in range(M2.shape[0]):
        bounds = np.flatnonzero(M2[b] > 0.5)
        bs = np.unique(np.concatenate([[0], bounds, [M2.shape[1]]]))
        ls = np.diff(bs)
        keep = ls > 0
        rows.append(np.full(keep.sum(), b))
        starts.append(bs[:-1][keep])
        lens.append(ls[keep])
    return (np.concatenate(rows), np.concatenate(starts), np.concatenate(lens))


def _tile_widths(k):
    """Split k slots into PE tiles of width {32,64,128}; width 96 is padded
    to 128 so every matmul output slice stays inside one PSUM bank."""
    ws = []
    while k > 0:
        if k >= 128:
            ws.append(128)
            k -= 128
        else:
            w = 32 * ((k + 31) // 32)
            ws.append(128 if w == 96 else w)
            k = 0
    return ws


def _schedule(all_lens):
    """Round schedule shared by all cores (SPMD: one compiled program)."""
    R = int(all_lens.max())
    widths, offs, off = [], [], 0
    for r in range(R):
        n_r = int((all_lens > r).sum())
        k_r = (n_r + N_CORES - 1) // N_CORES  # max episodes/core active
        ws = _tile_widths(k_r)
        widths.append(ws)
        offs.append(off)
        off += sum(ws)
    return R, widths, offs, off  # off == Npad


# ---------------------------------------------------------------------------
# bass program
# ---------------------------------------------------------------------------

# This neuronxcc build caps sync commands (waits + updates) per instruction
# by ISA struct size: plain ops like Drain/NoOp (CTRL_NO_STRUCT) get 1 slot,
# larger structs (matmul, tensor ops, ...) get 2.
_SMALL_OPS = {"Drain", "NoOp", "Nop", "EventSemaphoreOp", "SemaphoreOp"}


def _patch_bir_json(raw: bytes) -> bytes:
    """Hoist excess semaphore waits onto carrier NoOps inserted immediately
    before the over-cap instruction on the same engine — engines execute in
    order and Tile semaphores are monotonic, so this preserves semantics."""
    m = json.loads(raw)
    n = [0]
    for f in m["functions"]:
        for bb in f["blocks"]:
            out = []
            for inst in bb["instructions"]:
                si = inst.get("sync_info")
                waits = (si or {}).get("on_wait") or []
                upds = (si or {}).get("on_update") or []
                cap = 1 if inst.get("opcode") in _SMALL_OPS else 2
                keep = max(cap - len(upds), 0)
                if si and len(waits) > keep:
                    extra = waits[: len(waits) - keep]
                    si["on_wait"] = waits[len(waits) - keep :]
                    for w in extra:
                        n[0] += 1
                        out.append(
                            {
                                "name": f"I-syncw-{n[0]}",
                                "opcode": "Drain",
                                "engine": inst.get("engine", "SP"),
                                "ins": [],
                                "outs": [],
                                "debug": inst.get("debug", 0),
                                "sync_info": {"on_wait": [w], "on_update": []},
                            }
                        )
                out.append(inst)
            bb["instructions"] = out
    return json.dumps(m).encode()


def _build_bass(R, widths, offs, npad, s0cap, ncols0_tiles):
    import concourse.bass as bass
    import concourse.mybir as mybir
    import concourse.tile as tile

    f32 = mybir.dt.float32
    bf16 = mybir.dt.bfloat16
    SIG = mybir.ActivationFunctionType.Sigmoid
    TANH = mybir.ActivationFunctionType.Tanh

    nc = bass.Bass()
    obsT = nc.dram_tensor("obsT", [OBS_DIM + 1, npad], bf16, kind="ExternalInput")
    W1 = nc.dram_tensor("W1", [OBS_DIM + 1, 4 * HIDDEN], bf16, kind="ExternalInput")
    R2a = nc.dram_tensor("R2a", [128, 4 * HIDDEN], bf16, kind="ExternalInput")
    R2b = nc.dram_tensor("R2b", [128, 4 * HIDDEN], bf16, kind="ExternalInput")
    ih = nc.dram_tensor("ih", [128, GH, s0cap], bf16, kind="ExternalInput")
    ic = nc.dram_tensor("ic", [128, GH, s0cap], f32, kind="ExternalInput")
    HT = nc.dram_tensor("HT", [GH, 128, npad], bf16, kind="ExternalOutput")

    with tile.TileContext(nc) as tc:
        with (
            tc.tile_pool(name="const", bufs=1) as cpool,
            tc.tile_pool(name="state", bufs=1) as spool,
            tc.tile_pool(name="hseq", bufs=2) as hpool,
            tc.tile_pool(name="acts", bufs=2) as apool,
            tc.tile_pool(name="zp", bufs=3, space="PSUM") as zpool,
        ):
            w1s = cpool.tile([OBS_DIM + 1, 4 * HIDDEN], bf16)
            nc.sync.dma_start(out=w1s, in_=W1[:, :])
            r2a = cpool.tile([128, 4 * HIDDEN], bf16)
            nc.sync.dma_start(out=r2a, in_=R2a[:, :])
            r2b = cpool.tile([128, 4 * HIDDEN], bf16)
            nc.sync.dma_start(out=r2b, in_=R2b[:, :])
            obs = cpool.tile([OBS_DIM + 1, npad], bf16)
            nc.sync.dma_start(out=obs, in_=obsT[:, :])

            c_st = spool.tile([128, GH, s0cap], f32)
            nc.sync.dma_start(out=c_st, in_=ic[:, :, :])
            h_prev = hpool.tile([128, GH, s0cap], bf16, tag="h")
            nc.sync.dma_start(out=h_prev, in_=ih[:, :, :])

            for r in range(R):
                ws = widths[r]
                kp = sum(ws)
                off = offs[r]

                sig = apool.tile([128, 6, kp], bf16, tag="sig")
                tg = apool.tile([128, GH, kp], bf16, tag="tg")
                col = 0
                for w in ws:
                    z = zpool.tile([128, 8, w], f32, tag="z")
                    for j in range(8):
                        nc.tensor.matmul(
                            z[:, j, :],
                            w1s[:, j * 128 : (j + 1) * 128],
                            obs[:, off + col : off + col + w],
                            start=True,
                            stop=False,
                        )
                        nc.tensor.matmul(
                            z[:, j, :],
                            r2a[:, j * 128 : (j + 1) * 128],
                            h_prev[:, 0, col : col + w],
                            start=False,
                            stop=False,
                        )
                        nc.tensor.matmul(
                            z[:, j, :],
                            r2b[:, j * 128 : (j + 1) * 128],
                            h_prev[:, 1, col : col + w],
                            start=False,
                            stop=True,
                        )
                    nc.scalar.activation(sig[:, :, col : col + w], z[:, 0:6, :], SIG)
                    nc.scalar.activation(tg[:, :, col : col + w], z[:, 6:8, :], TANH)
                    col += w

                ig = apool.tile([128, GH, kp], bf16, tag="ig")
                nc.vector.tensor_mul(ig, sig[:, 0:2, :], tg)
                fc = apool.tile([128, GH, kp], f32, tag="fc")
                nc.vector.tensor_mul(fc, sig[:, 2:4, :], c_st[:, :, 0:kp])
                nc.vector.tensor_add(c_st[:, :, 0:kp], ig, fc)
                tc_t = apool.tile([128, GH, kp], bf16, tag="tc")
                nc.scalar.activation(tc_t, c_st[:, :, 0:kp], TANH)
                h_new = hpool.tile([128, GH, kp], bf16, tag="h")
                nc.vector.tensor_mul(h_new, sig[:, 4:6, :], tc_t)
                nc.sync.dma_start(
                    out=HT[:, :, off : off + kp].rearrange("g p n -> p g n"),
                    in_=h_new,
                )
                h_prev = h_new

    orig = nc.to_json_bytes
    nc.to_json_bytes = lambda: _patch_bir_json(orig())
    return nc


# ---------------------------------------------------------------------------
# entry point
# ---------------------------------------------------------------------------


LAST_RESULT = None  # BassKernelResults of the most recent run (for profiling)


def kernel(obs, S, M, W_embed, b_embed, kernel, rec_kernel, bias):
    import os

    from concourse.bass_utils import run_bass_kernel_spmd

    obs = np.asarray(obs, np.float32)
    S = np.asarray(S, np.float32)
    M = np.asarray(M, np.float32)
    W_embed = np.asarray(W_embed, np.float32)
    b_embed = np.asarray(b_embed, np.float32)
    kernel_w = np.asarray(kernel, np.float32)
    rec_kernel = np.asarray(rec_kernel, np.float32)
    bias = np.asarray(bias, np.float32)

    B = S.shape[0]
    T = obs.shape[0] // B
    H = HIDDEN

    # ---- weights: fold embedding, add bias row, permute gates to [i|f|o|g]
    perm = np.concatenate(
        [np.arange(0, 2 * H), np.arange(3 * H, 4 * H), np.arange(2 * H, 3 * H)]
    )
    w1 = np.vstack(
        [W_embed @ kernel_w, (b_embed @ kernel_w + bias)[None, :]]
    )[:, perm]
    r2 = rec_kernel[:, perm]

    # ---- episode segmentation + schedule
    M2 = M.reshape(B, T)
    rows, starts, lens = _episodes(M2)
    order = np.argsort(-lens, kind="stable")
    rows, starts, lens = rows[order], starts[order], lens[order]
    R, widths, offs, npad = _schedule(lens)
    s0cap = sum(widths[0])
    ncols0_tiles = len(widths[0])

    # per-core episode lists (round-robin over the sorted order)
    in_maps = []
    scat_src = []  # per core: packed column -> memory row
    scat_col = []
    for c in range(N_CORES):
        er = rows[c::N_CORES]
        es = starts[c::N_CORES]
        el = lens[c::N_CORES]
        ne = len(el)

        obs1 = np.zeros((OBS_DIM + 1, npad), np.float32)
        obs1[OBS_DIM, :] = 1.0
        cols_all, srcs_all = [], []
        for r in range(R):
            k = int((el > r).sum())  # prefix (sorted desc)
            if k == 0:
                continue
            cols = offs[r] + np.arange(k)
            srcs = er[:k] * T + es[:k] + r
            cols_all.append(cols)
            srcs_all.append(srcs)
        cols_all = np.concatenate(cols_all)
        srcs_all = np.concatenate(srcs_all)
        obs1[:OBS_DIM, cols_all] = obs[srcs_all].T

        ih = np.zeros((128, GH, s0cap), np.float32)
        ic = np.zeros((128, GH, s0cap), np.float32)
        first = es == 0  # episodes that inherit S[row]
        idx = np.flatnonzero(first)
        if len(idx):
            for g in range(GH):
                ih[:, g, idx] = S[er[idx], g * 128 : (g + 1) * 128].T
                ic[:, g, idx] = S[er[idx], H + g * 128 : H + (g + 1) * 128].T

        in_maps.append(
            {
                "obsT": obs1.astype(BF16),
                "W1": w1.astype(BF16),
                "R2a": r2[:128].astype(BF16),
                "R2b": r2[128:].astype(BF16),
                "ih": ih.astype(BF16),
                "ic": ic.astype(np.float32),
            }
        )
        scat_src.append(srcs_all)
        scat_col.append(cols_all)

    nc = _build_bass(R, widths, offs, npad, s0cap, ncols0_tiles)
    trace = bool(int(os.environ.get("KERNEL_TRACE", "0")))
    res = run_bass_kernel_spmd(
        nc, in_maps, core_ids=list(range(N_CORES)), trace=trace
    )
    global LAST_RESULT
    LAST_RESULT = res

    memory = np.zeros((B * T, H), np.float32)
    for c in range(N_CORES):
        ht = np.asarray(res.results[c]["HT"]).astype(np.float32)  # [GH,128,npad]
        packed = ht.transpose(2, 0, 1).reshape(npad, H)  # [npad, H]
        memory[scat_src[c]] = packed[scat_col[c]]
    return memory


# revision 11
# speedup vs baseline: 1.1415x; 1.1415x over previous
# Masked-reset LSTM (MemoryEmbedding) Trainium2 kernel.
#
# Strategy: the episode-reset mask M (binary) splits every batch row into
# independent episodes, each starting from a known state (S[row] at t=0,
# zeros at every reset).  On the host we segment all B*T steps into
# episodes, sort them by length (desc) and deal them round-robin onto the
# 8 cores.  The device then runs a "wavefront" over relative step r
# (round r processes step r of every episode longer than r); because the
# episodes are sorted, the active set at every round is a prefix of the
# slot axis, so all device work is dense.  Sequential depth drops from
# T=2048 to max-episode-length (~60-90 for 10% reset rate).
#
# Each core's episodes are further split into two interleaved groups (A/B)
# with independent state; the two dependency chains overlap on the engines,
# hiding the per-round semaphore/issue latency that dominates the tail.
#
# Device layout is fully transposed (gate/hidden units on partitions,
# episode slots on the free axis):
#   z^T[j*128:(j+1)*128, slot] = W1[:, j]^T @ obs1^T + R2[:, j]^T @ h^T
# with W1 = [W_embed @ kernel ; b_embed @ kernel + bias] (65 rows: 64 obs
# dims + ones-row for the bias, zero-padded to 128) and gate columns
# permuted to [i|f|o|g] so one sigmoid covers blocks 0..5 and one tanh
# covers blocks 6..7.

import json

import ml_dtypes
import numpy as np

OBS_DIM, EMBED_DIM, HIDDEN = 64, 128, 256
BATCH, SEQLEN = 32, 2048
N_CORES = 8
N_GROUPS = 2
GH = HIDDEN // 128  # 2 gate-partition tiles per gate
KDIM = 128  # contraction rows for the obs-side matmul (64 obs + 1 bias + pad)

BF16 = ml_dtypes.bfloat16

# ---------------------------------------------------------------------------
# host-side helpers
# ---------------------------------------------------------------------------


def _episodes(M2):
    """M2: [B, T] binary mask -> arrays (row, start, length), unsorted."""
    rows, starts, lens = [], [], []
    for b in range(M2.shape[0]):
        bounds = np.flatnonzero(M2[b] > 0.5)
        bs = np.unique(np.concatenate([[0], bounds, [M2.shape[1]]]))
        ls = np.diff(bs)
        keep = ls > 0
        rows.append(np.full(keep.sum(), b))
        starts.append(bs[:-1][keep])
        lens.append(ls[keep])
    return (np.concatenate(rows), np.concatenate(starts), np.concatenate(lens))


def _tile_widths(k):
    """Split k slots into PE tiles; widths chosen so every matmul output
    slice [128, w] at col j*w stays inside one 2KB PSUM bank."""
    ws = []
    while k > 0:
        if k >= 128:
            ws.append(128)
            k -= 128
        else:
            w = 16 * ((k + 15) // 16)
            ws.append(128 if w in (80, 96, 112) else w)
            k = 0
    return ws


# ---------------------------------------------------------------------------
# bass program
# ---------------------------------------------------------------------------

# This neuronxcc build caps sync commands (waits + updates) per instruction
# by ISA struct size: plain ops like Drain (CTRL_NO_STRUCT) get 1 slot,
# larger structs (matmul, tensor ops, ...) get 2.
_SMALL_OPS = {"Drain", "NoOp", "Nop", "EventSemaphoreOp", "SemaphoreOp"}


def _patch_bir_json(raw: bytes) -> bytes:
    """Hoist excess semaphore waits onto carrier Drains inserted immediately
    before the over-cap instruction on the same engine — engines execute in
    order and Tile semaphores are monotonic, so this preserves semantics."""
    m = json.loads(raw)
    n = [0]
    for f in m["functions"]:
        for bb in f["blocks"]:
            out = []
            for inst in bb["instructions"]:
                si = inst.get("sync_info")
                waits = (si or {}).get("on_wait") or []
                upds = (si or {}).get("on_update") or []
                cap = 1 if inst.get("opcode") in _SMALL_OPS else 2
                keep = max(cap - len(upds), 0)
                if si and len(waits) > keep:
                    extra = waits[: len(waits) - keep]
                    si["on_wait"] = waits[len(waits) - keep :]
                    for w in extra:
                        n[0] += 1
                        out.append(
                            {
                                "name": f"I-syncw-{n[0]}",
                                "opcode": "Drain",
                                "engine": inst.get("engine", "SP"),
                                "ins": [],
                                "outs": [],
                                "debug": inst.get("debug", 0),
                                "sync_info": {"on_wait": [w], "on_update": []},
                            }
                        )
                out.append(inst)
            bb["instructions"] = out
    return json.dumps(m).encode()


def _build_bass(R, scheds, npad, s0caps):
    """scheds: per group dict with widths[r] (list, maybe empty), offs[r]."""
    import concourse.bass as bass
    import concourse.mybir as mybir
    import concourse.tile as tile

    f32 = mybir.dt.float32
    bf16 = mybir.dt.bfloat16
    SIG = mybir.ActivationFunctionType.Sigmoid
    TANH = mybir.ActivationFunctionType.Tanh

    s0tot = sum(s0caps)
    nc = bass.Bass()
    obsT = nc.dram_tensor("obsT", [KDIM, npad], bf16, kind="ExternalInput")
    W1 = nc.dram_tensor("W1", [KDIM, 4 * HIDDEN], bf16, kind="ExternalInput")
    R2a = nc.dram_tensor("R2a", [128, 4 * HIDDEN], bf16, kind="ExternalInput")
    R2b = nc.dram_tensor("R2b", [128, 4 * HIDDEN], bf16, kind="ExternalInput")
    ih = nc.dram_tensor("ih", [128, GH, s0tot], bf16, kind="ExternalInput")
    ic = nc.dram_tensor("ic", [128, GH, s0tot], f32, kind="ExternalInput")
    HT = nc.dram_tensor("HT", [GH, 128, npad], bf16, kind="ExternalOutput")

    with tile.TileContext(nc) as tc:
        with (
            tc.tile_pool(name="const", bufs=1) as cpool,
            tc.tile_pool(name="state", bufs=1) as spool,
            tc.tile_pool(name="hseq", bufs=3) as hpool,
            tc.tile_pool(name="acts", bufs=3) as apool,
            tc.tile_pool(name="zp", bufs=4, space="PSUM") as zpool,
        ):
            w1s = cpool.tile([KDIM, 4 * HIDDEN], bf16)
            nc.sync.dma_start(out=w1s, in_=W1[:, :])
            r2a = cpool.tile([128, 4 * HIDDEN], bf16)
            nc.sync.dma_start(out=r2a, in_=R2a[:, :])
            r2b = cpool.tile([128, 4 * HIDDEN], bf16)
            nc.sync.dma_start(out=r2b, in_=R2b[:, :])
            obs = cpool.tile([KDIM, npad], bf16)
            nc.sync.dma_start(out=obs, in_=obsT[:, :])

            c_st = []
            h_prev = []
            goff = 0
            for g in range(N_GROUPS):
                cs = spool.tile(
                    [128, GH, s0caps[g]], f32, name=f"c_st{g}", tag=f"c{g}"
                )
                nc.sync.dma_start(out=cs, in_=ic[:, :, goff : goff + s0caps[g]])
                hp = hpool.tile(
                    [128, GH, s0caps[g]], bf16, name=f"h0_{g}", tag=f"h{g}"
                )
                nc.sync.dma_start(out=hp, in_=ih[:, :, goff : goff + s0caps[g]])
                c_st.append(cs)
                h_prev.append(hp)
                goff += s0caps[g]

            for r in range(R):
                for g in range(N_GROUPS):
                    ws = scheds[g]["widths"][r] if r < len(scheds[g]["widths"]) else []
                    if not ws:
                        continue
                    kp = sum(ws)
                    off = scheds[g]["offs"][r]
                    hp = h_prev[g]
                    cs = c_st[g]

                    sig = apool.tile(
                        [128, 6, kp], bf16, name=f"sig{g}", tag=f"sig{g}"
                    )
                    tg = apool.tile([128, GH, kp], bf16, name=f"tg{g}", tag=f"tg{g}")
                    col = 0
                    for w in ws:
                        z = zpool.tile([128, 8, w], f32, name=f"z{g}", tag="z")
                        for j in range(8):
                            nc.tensor.matmul(
                                z[:, j, :],
                                w1s[:, j * 128 : (j + 1) * 128],
                                obs[:, off + col : off + col + w],
                                start=True,
                                stop=False,
                            )
                            nc.tensor.matmul(
                                z[:, j, :],
                                r2a[:, j * 128 : (j + 1) * 128],
                                hp[:, 0, col : col + w],
                                start=False,
                                stop=False,
                            )
                            nc.tensor.matmul(
                                z[:, j, :],
                                r2b[:, j * 128 : (j + 1) * 128],
                                hp[:, 1, col : col + w],
                                start=False,
                                stop=True,
                            )
                        nc.scalar.activation(
                            sig[:, :, col : col + w], z[:, 0:6, :], SIG
                        )
                        nc.scalar.activation(
                            tg[:, :, col : col + w], z[:, 6:8, :], TANH
                        )
                        col += w

                    ig = apool.tile([128, GH, kp], bf16, name=f"ig{g}", tag=f"ig{g}")
                    nc.vector.tensor_mul(ig, sig[:, 0:2, :], tg)
                    fc = apool.tile([128, GH, kp], f32, name=f"fc{g}", tag=f"fc{g}")
                    nc.gpsimd.tensor_mul(fc, sig[:, 2:4, :], cs[:, :, 0:kp])
                    nc.vector.tensor_add(cs[:, :, 0:kp], ig, fc)
                    tc_t = apool.tile(
                        [128, GH, kp], bf16, name=f"tc{g}", tag=f"tc{g}"
                    )
                    nc.scalar.activation(tc_t, cs[:, :, 0:kp], TANH)
                    h_new = hpool.tile(
                        [128, GH, kp], bf16, name=f"h{g}", tag=f"h{g}"
                    )
                    nc.vector.tensor_mul(h_new, sig[:, 4:6, :], tc_t)
                    nc.sync.dma_start(
                        out=HT[:, :, off : off + kp].rearrange("g p n -> p g n"),
                        in_=h_new,
                    )
                    h_prev[g] = h_new

    orig = nc.to_json_bytes
    nc.to_json_bytes = lambda: _patch_bir_json(orig())
    return nc


# ---------------------------------------------------------------------------
# entry point
# ---------------------------------------------------------------------------

LAST_RESULT = None  # BassKernelResults of the most recent run (for profiling)


def kernel(obs, S, M, W_embed, b_embed, kernel, rec_kernel, bias):
    import os

    from concourse.bass_utils import run_bass_kernel_spmd

    obs = np.asarray(obs, np.float32)
    S = np.asarray(S, np.float32)
    M = np.asarray(M, np.float32)
    W_embed = np.asarray(W_embed, np.float32)
    b_embed = np.asarray(b_embed, np.float32)
    kernel_w = np.asarray(kernel, np.float32)
    rec_kernel = np.asarray(rec_kernel, np.float32)
    bias = np.asarray(bias, np.float32)

    B = S.shape[0]
    T = obs.shape[0] // B
    H = HIDDEN

    # ---- weights: fold embedding, add bias row, permute gates to [i|f|o|g]
    perm = np.concatenate(
        [np.arange(0, 2 * H), np.arange(3 * H, 4 * H), np.arange(2 * H, 3 * H)]
    )
    w1 = np.zeros((KDIM, 4 * H), np.float32)
    w1[:OBS_DIM] = (W_embed @ kernel_w)[:, perm]
    w1[OBS_DIM] = (b_embed @ kernel_w + bias)[perm]
    r2 = rec_kernel[:, perm]

    # ---- episode segmentation
    M2 = M.reshape(B, T)
    rows, starts, lens = _episodes(M2)
    order = np.argsort(-lens, kind="stable")
    rows, starts, lens = rows[order], starts[order], lens[order]
    R = int(lens.max())

    # per (core, group) episode lists: core c takes sorted ranks c::8, then
    # alternates its local list between the two groups.
    eps = {}  # (c, g) -> (row, start, len), sorted desc by len
    for c in range(N_CORES):
        er, es, el = rows[c::N_CORES], starts[c::N_CORES], lens[c::N_CORES]
        for g in range(N_GROUPS):
            eps[(c, g)] = (er[g::N_GROUPS], es[g::N_GROUPS], el[g::N_GROUPS])

    # shared schedules (one compiled SPMD program): per group, per round,
    # the max active count over cores, tiled into PE widths.
    scheds = []
    off = 0
    offs_flat = {}
    for g in range(N_GROUPS):
        widths, offs = [], []
        for r in range(R):
            k = max(int((eps[(c, g)][2] > r).sum()) for c in range(N_CORES))
            if k == 0:
                break
            ws = _tile_widths(k)
            widths.append(ws)
            offs.append(off)
            offs_flat[(g, r)] = off
            off += sum(ws)
        scheds.append({"widths": widths, "offs": offs})
    npad = off
    s0caps = [sum(scheds[g]["widths"][0]) for g in range(N_GROUPS)]

    # ---- per-core packed inputs
    in_maps = []
    scat_src = []
    scat_col = []
    for c in range(N_CORES):
        obs1 = np.zeros((KDIM, npad), np.float32)
        obs1[OBS_DIM, :] = 1.0
        cols_all, srcs_all = [], []
        ihb = np.zeros((128, GH, sum(s0caps)), np.float32)
        icb = np.zeros((128, GH, sum(s0caps)), np.float32)
        goff = 0
        for g in range(N_GROUPS):
            er, es, el = eps[(c, g)]
            for r in range(len(scheds[g]["widths"])):
                k = int((el > r).sum())
                if k == 0:
                    break
                cols = offs_flat[(g, r)] + np.arange(k)
                srcs = er[:k] * T + es[:k] + r
                cols_all.append(cols)
                srcs_all.append(srcs)
            idx = np.flatnonzero(es == 0)  # episodes inheriting S[row]
            for gg in range(GH):
                ihb[:, gg, goff + idx] = S[er[idx], gg * 128 : (gg + 1) * 128].T
                icb[:, gg, goff + idx] = S[
                    er[idx], H + gg * 128 : H + (gg + 1) * 128
                ].T
            goff += s0caps[g]
        cols_all = np.concatenate(cols_all)
        srcs_all = np.concatenate(srcs_all)
        obs1[:OBS_DIM, cols_all] = obs[srcs_all].T

        in_maps.append(
            {
                "obsT": obs1.astype(BF16),
                "W1": w1.astype(BF16),
                "R2a": r2[:128].astype(BF16),
                "R2b": r2[128:].astype(BF16),
                "ih": ihb.astype(BF16),
                "ic": icb.astype(np.float32),
            }
        )
        scat_src.append(srcs_all)
        scat_col.append(cols_all)

    nc = _build_bass(R, scheds, npad, s0caps)
    trace = bool(int(os.environ.get("KERNEL_TRACE", "0")))
    res = run_bass_kernel_spmd(
        nc, in_maps, core_ids=list(range(N_CORES)), trace=trace
    )
    global LAST_RESULT
    LAST_RESULT = res

    memory = np.zeros((B * T, H), np.float32)
    for c in range(N_CORES):
        ht = np.asarray(res.results[c]["HT"]).astype(np.float32)  # [GH,128,npad]
        packed = ht.transpose(2, 0, 1).reshape(npad, H)
        memory[scat_src[c]] = packed[scat_col[c]]
    return memory
